# revision 1
# baseline (speedup 1.0000x reference)
"""Trainium2 Bass kernel for a 3D windowed-attention transformer block.

Data-parallel over the 16 attention windows (2 windows/core x 8 cores).
Device tensors live in C-layout [channels(partitions), tokens(free)].

Key design points vs the earlier baseline:
- Weight DMAs are issued ahead of use into non-aliasing pools (attention
  weights at t=0, MLP weights as soon as attention SBUF frees), so the
  PE never stalls on SBUF reuse.
- x ships as bf16; LayerNorm runs with PE stats/broadcasts, DVE subs and
  GPSIMD (SBUF-only) multiplies.
- The decomposed rel-pos bias scatter uses per-offset shift-selector
  matmuls (N=64, M=8) into 32-aligned PSUM slots, evacuated by aligned
  copies (d goes straight to qhat; w/h via one bf16 staging tile and two
  identity SBUF-to-SBUF DMAs per head). This replaces the baseline's 192
  gpsimd DMAs + 96 N=512 selector matmuls + 384 small DVE copies.
- qkv bias is accumulated in PSUM via a K=1 rank-1 matmul; the ACT
  engine mostly runs exp/gelu plus a few evacuations.
- The attention PV contraction and the output projection run in
  fp8-e4m3 DoubleRow (exp output bounded well inside e4m3 range; proj
  weights prescaled x64 and ctx x16 to escape the e4m3 subnormal zone,
  compensated by a 1/1024 scale folded into the residual add). vhat
  ko-stride padded to 784 for the 16B DoubleRow alignment rule. The MLP
  stays bf16: fp8 there costs ~4% relative error on the MLP branch
  (~2e-2 end-to-end), over the gate.
- Hardware legality notes baked in: GPSIMD cannot touch PSUM; compute
  ops need one PSUM operand max and 32-aligned partition bases; DMA APs
  cross partitions only via their first dim; matmul PSUM base must equal
  tile_position[1] (explicit for the 96-base slot).
"""

import ml_dtypes
import numpy as np

import concourse.bass as bass
import concourse.mybir as mybir
import concourse.tile as tile
from concourse import bacc

DIM = 768
NH = 12
HD = 64
WS = 8
NTOK = 1024          # tokens per core (2 windows x 512)
NWIN = 2
KT = DIM // 128      # 6 k-tiles over channels
SCALE = HD ** -0.5
EPS = 1e-5
F32 = mybir.dt.float32
F32R = mybir.dt.float32r
BF16 = mybir.dt.bfloat16
F8 = mybir.dt.float8e4

Ident = mybir.ActivationFunctionType.Identity
Exp = mybir.ActivationFunctionType.Exp
Gelu = mybir.ActivationFunctionType.Gelu
Sigmoid = mybir.ActivationFunctionType.Sigmoid
Sqrt = mybir.ActivationFunctionType.Sqrt
Rsqrt = mybir.ActivationFunctionType.Rsqrt
ADD = mybir.AluOpType.add
DR = mybir.MatmulPerfMode.DoubleRow

# CoreSim has no Gelu; set True (sim only) to use x*sigmoid(1.702x) instead
SIM_GELU = False


# ---------------------------------------------------------------------------
# device program
# ---------------------------------------------------------------------------

def _emit_ln(nc, tc, xs, write_out, consts):
    """Standardize along channels (partition dim): out = (x - mean)*rstd.

    xs: 6 input tiles [128, 1024] (bf16 or fp32).
    write_out(k, ch, sb_ap, bc_r_ap): final multiply destination hook.
    """
    from contextlib import ExitStack
    with ExitStack() as ctx:
        stat_ps = ctx.enter_context(tc.tile_pool(name="ln_stat_ps", bufs=2, space="PSUM"))
        bc_ps = ctx.enter_context(tc.tile_pool(name="ln_bc_ps", bufs=2, space="PSUM"))
        sq_pool = ctx.enter_context(tc.tile_pool(name="ln_sq", bufs=2))
        st_pool = ctx.enter_context(tc.tile_pool(name="ln_st", bufs=2))
        sb_pool = ctx.enter_context(tc.tile_pool(name="ln_sb", bufs=4))
        for ch in range(2):
            cols = bass.ds(ch * 512, 512)
            ps_s = stat_ps.tile([1, 512], F32, tag="ps_s")
            ps_q = stat_ps.tile([1, 512], F32, tag="ps_q")
            for k in range(KT):
                sq = sq_pool.tile([128, 512], BF16, tag="sq")
                nc.gpsimd.tensor_mul(sq, xs[k][:, cols], xs[k][:, cols])
                nc.tensor.matmul(ps_s, consts["ones_col"], xs[k][:, cols],
                                 start=(k == 0), stop=(k == KT - 1))
                nc.tensor.matmul(ps_q, consts["ones_col"], sq,
                                 start=(k == 0), stop=(k == KT - 1))
            mean = st_pool.tile([1, 512], BF16, tag="mean")
            ex2 = st_pool.tile([1, 512], F32, tag="ex2")
            nc.scalar.mul(mean, ps_s, 1.0 / DIM)
            nc.scalar.mul(ex2, ps_q, 1.0 / DIM)
            mm = st_pool.tile([1, 512], F32, tag="mm")
            nc.vector.tensor_mul(mm, mean, mean)
            var = st_pool.tile([1, 512], F32, tag="var")
            nc.vector.tensor_sub(var, ex2, mm)
            rstd = st_pool.tile([1, 512], F32, tag="rstd")
            nc.scalar.activation(rstd, var, Sqrt,
                                 bias=consts["eps"][0:1, 0:1], scale=1.0)
            rstd_b = st_pool.tile([1, 512], BF16, tag="rstd_b")
            nc.vector.reciprocal(rstd_b, rstd)
            bc_m = bc_ps.tile([128, 512], F32, tag="bc_m")
            bc_r = bc_ps.tile([128, 512], F32, tag="bc_r")
            nc.tensor.matmul(bc_m, consts["ones_row"], mean, start=True, stop=True)
            nc.tensor.matmul(bc_r, consts["ones_row"], rstd_b, start=True, stop=True)
            # gpsimd cannot read PSUM: evacuate bc_r once, then the final
            # multiplies run SBUF-only on gpsimd while DVE does the subs
            bc_rs = sb_pool.tile([128, 512], BF16, tag="bc_rs")
            nc.scalar.copy(bc_rs, bc_r)
            for k in range(KT):
                sb = sb_pool.tile([128, 512], BF16, tag="sb")
                nc.vector.tensor_sub(sb, xs[k][:, cols], bc_m)
                write_out(k, ch, sb, bc_rs)


def _emit(nc, tc, d):
    """Emit the whole per-core program. d: dict of DRAM APs."""
    from contextlib import ExitStack

    with ExitStack() as top:
        top.enter_context(nc.allow_low_precision(
            reason="bf16/fp8 matmul operands are intentional; accumulation stays fp32"))

        # ---- persistent constants & weights; DMAs all start here ----
        consts_pool = top.enter_context(tc.tile_pool(name="consts", bufs=1))
        ones_col = consts_pool.tile([128, 1], BF16, tag="ones_col")
        nc.vector.memset(ones_col, 1.0)

        ones_row = consts_pool.tile([1, 128], BF16, tag="ones_row")
        nc.vector.memset(ones_row, 1.0)
        ones_r64 = consts_pool.tile([1, 64], BF16, tag="ones_r64")
        nc.vector.memset(ones_r64, 1.0)
        ones512 = consts_pool.tile([1, 512], BF16, tag="ones512")
        nc.vector.memset(ones512, 1.0)
        eps = consts_pool.tile([1, 1], F32, tag="eps")
        nc.vector.memset(eps, EPS)
        inv1024 = consts_pool.tile([128, 1], F32, tag="inv1024")
        nc.vector.memset(inv1024, 1.0 / 1024.0)
        c16 = consts_pool.tile([128, 1], F32, tag="c16")
        nc.vector.memset(c16, 16.0)
        consts = dict(ones_col=ones_col, ones_row=ones_row, eps=eps)

        # x input first (LN1 needs it), then weights in order of use
        xs_pool = top.enter_context(tc.tile_pool(name="xs", bufs=KT))
        xs = [xs_pool.tile([128, NTOK], BF16, tag="xs", name=f"xs{i}") for i in range(KT)]
        for k in range(KT):
            nc.sync.dma_start(out=xs[k], in_=d["xT"][k * 128:(k + 1) * 128, :])

        e2_sb = consts_pool.tile([64, NTOK], BF16, tag="e2")
        nc.sync.dma_start(out=e2_sb, in_=d["e2"])
        shsel = consts_pool.tile([128, 384], BF16, tag="shsel")
        nc.sync.dma_start(out=shsel, in_=d["shsel"])
        baug = consts_pool.tile([1, 2304], BF16, tag="baug")
        nc.sync.dma_start(out=baug, in_=d["baug"])
        bv = consts_pool.tile([1, 768], BF16, tag="bv")
        nc.sync.dma_start(out=bv, in_=d["bv"])
        bp = consts_pool.tile([128, 6], F32, tag="bp")
        nc.sync.dma_start(out=bp, in_=d["bp"])
        b1 = consts_pool.tile([128, 24], F32, tag="b1")
        nc.sync.dma_start(out=b1, in_=d["b1"])
        b2 = consts_pool.tile([128, 6], F32, tag="b2")
        nc.sync.dma_start(out=b2, in_=d["b2"])

        # later-phase weights: pools persistent (top), DMAs issued after the
        # attention-phase weights below (queue order = call order)
        wp_pool = top.enter_context(tc.tile_pool(name="wp", bufs=1))
        wpt = wp_pool.tile([128, 3 * 2 * 768], F8, tag="wpt")
        wpv = wpt.rearrange("p (g ko m) -> p g ko m", g=3, ko=2)

        # ---- scoped activation tensors (manual LIFO: enter ctx, then
        # attention-phase pools; exit attention pools in reverse, ctx after
        # proj) ----
        ctx_cm = tc.tile_pool(name="ctxT", bufs=3)
        ctx_pool = ctx_cm.__enter__()
        ctxT3 = [ctx_pool.tile([128, 2 * NTOK], F8, tag="ctxT", name=f"ctxT{i}") for i in range(3)]
        ctxv = [t.rearrange("p (ko n) -> p ko n", ko=2) for t in ctxT3]

        wq_cm = tc.tile_pool(name="wqfull", bufs=KT)
        qh_cm = tc.tile_pool(name="qhat", bufs=NH)
        kh_cm = tc.tile_pool(name="khat", bufs=NH)
        vh_cm = tc.tile_pool(name="vhat", bufs=4)
        xh_cm = tc.tile_pool(name="xhat", bufs=KT)
        wq_pool = wq_cm.__enter__()
        qh_pool = qh_cm.__enter__()
        kh_pool = kh_cm.__enter__()
        vh_pool = vh_cm.__enter__()
        xh_pool = xh_cm.__enter__()
        wqs = [wq_pool.tile([128, 2304], BF16, tag="wqf", name=f"wqf{i}") for i in range(KT)]
        qhat = [qh_pool.tile([128, NTOK], BF16, tag="qhat", name=f"qhat{i}") for i in range(NH)]
        khat = [kh_pool.tile([128, NTOK], BF16, tag="khat", name=f"khat{i}") for i in range(NH)]
        vhat = [vh_pool.tile([128, 2 * 784], F8, tag="vhat", name=f"vhat{i}") for i in range(4)]
        xhat = [xh_pool.tile([128, NTOK], BF16, tag="xh", name=f"xh{i}") for i in range(KT)]

        # v weights: scoped pool (freed right after the v projection);
        # DMA goes on the scalar queue so the sync queue keeps priority order
        wv_cm = tc.tile_pool(name="wv", bufs=KT)
        wv_pool = wv_cm.__enter__()
        wvs = [wv_pool.tile([128, 768], BF16, tag="wv", name=f"wv{i}") for i in range(KT)]
        for k in range(KT):
            nc.scalar.dma_start(out=wvs[k], in_=d["wv"][k * 128:(k + 1) * 128, :])

        # weight DMAs on sync, in order of first use
        for k in range(KT):
            nc.sync.dma_start(out=wqs[k], in_=d["waug"][k * 128:(k + 1) * 128, :])
        nc.sync.dma_start(out=wpt, in_=d["wp"])

        # khat one-hot key-position rows; qhat memset (rows 16-31/40-63 must be
        # finite; bias rows and q rows get overwritten)
        for h in range(NH):
            nc.sync.dma_start(out=khat[h][0:64, :], in_=d["e2"])
            nc.gpsimd.memset(qhat[h], 0.0)
        for t in range(4):
            for ko in range(2):
                vv = vhat[t][:, ko * 784: ko * 784 + 780].rearrange(
                    "p (h c) -> p h c", c=65)
                nc.vector.memset(vv[:, :, 64:65], 1.0)

        # ---- LN1 -> xhat (bf16) ----
        def ln1_out(k, ch, sb, bc_rs):
            nc.gpsimd.tensor_mul(xhat[k][:, bass.ds(ch * 512, 512)], sb, bc_rs)
        _emit_ln(nc, tc, xs, ln1_out, consts)

        # ---- v projection -> vhat (fp8 pairs for DoubleRow PV) ----
        with ExitStack() as vph:
            v_ps = vph.enter_context(tc.tile_pool(name="v_ps", bufs=2, space="PSUM"))
            for t in range(8):
                for nch in range(2):
                    pv = v_ps.tile([128, 384], F32, tag="vps")
                    for k in range(KT):
                        nc.tensor.matmul(pv, xhat[k][:, t * 128:(t + 1) * 128],
                                         wvs[k][:, nch * 384:(nch + 1) * 384],
                                         start=(k == 0), stop=False)
                    nc.tensor.matmul(pv, ones_row, bv[0:1, nch * 384:(nch + 1) * 384],
                                     start=False, stop=True)
                    vv = vhat[t // 2][:, (t % 2) * 784 + nch * 390:
                                      (t % 2) * 784 + (nch + 1) * 390].rearrange(
                        "p (h c) -> p h c", c=65)
                    dst = vv[:, :, 0:64]
                    src = pv.rearrange("p (h c) -> p h c", c=64)
                    if t % 2 == 0:
                        nc.vector.tensor_copy(dst, src)
                    else:
                        nc.scalar.copy(dst, src)
        wv_cm.__exit__(None, None, None)

        # ---- qkv + rel-pos scatter + attention, per head-pair ----
        with ExitStack() as ph:
            qkv_ps = ph.enter_context(tc.tile_pool(name="qkv_ps", bufs=2, space="PSUM"))
            pb_ps = ph.enter_context(tc.tile_pool(name="pb_ps", bufs=1, space="PSUM"))
            psb_pool = ph.enter_context(tc.tile_pool(name="psb", bufs=2))
            s_ps = ph.enter_context(tc.tile_pool(name="s_ps", bufs=2, space="PSUM"))
            c_ps = ph.enter_context(tc.tile_pool(name="c_ps", bufs=1, space="PSUM"))
            r_ps = ph.enter_context(tc.tile_pool(name="r_ps", bufs=1, space="PSUM"))
            pt_pool = ph.enter_context(tc.tile_pool(name="ptile", bufs=2))
            dn_pool = ph.enter_context(tc.tile_pool(name="dn", bufs=2))

            for hp in range(6):
                for m in (hp, 6 + hp, 12 + hp):
                    for ch in range(2):
                        cols = bass.ds(ch * 512, 512)
                        pt = qkv_ps.tile([128, 512], F32, tag="qkvps",
                                         name=f"qkvps_{m}_{ch}")
                        for k in range(KT):
                            nc.tensor.matmul(
                                pt, wqs[k][:, m * 128:(m + 1) * 128],
                                xhat[k][:, cols], start=(k == 0), stop=False)
                        nc.tensor.matmul(pt, baug[0:1, m * 128:(m + 1) * 128],
                                         ones512, start=False, stop=True)
                        if m < 6:        # q rows (scaled): heads 2m, 2m+1
                            nc.vector.tensor_copy(qhat[2 * m][64:128, cols], pt[0:64, :])
                            nc.vector.tensor_copy(qhat[2 * m + 1][64:128, cols], pt[64:128, :])
                        elif m < 12:     # k rows: heads 2(m-6), 2(m-6)+1
                            nc.vector.tensor_copy(khat[2 * (m - 6)][64:128, cols], pt[0:64, :])
                            nc.vector.tensor_copy(khat[2 * (m - 6) + 1][64:128, cols], pt[64:128, :])
                        else:            # P rows -> shift-selector scatter
                            psb = psb_pool.tile([128, 512], BF16, tag="psb",
                                                name=f"psb_{m}_{ch}")
                            nc.vector.tensor_copy(psb, pt)
                            # shift matmuls write 6 (hh, table) slots at
                            # 32-aligned PSUM rows across 2 banks; cols are
                            # dlt*64 + stream position.
                            pbA = pb_ps.tile([128, 512], F32, tag="pbA",
                                             name=f"pbA_{m}_{ch}")
                            pbB = pb_ps.tile([64, 512], F32, tag="pbB",
                                             name=f"pbB_{m}_{ch}")
                            # slot -> (tile, row): (hh, ti): ti 0=d, 1=w, 2=h
                            slots = {(0, 0): (pbA, 0), (0, 1): (pbA, 32),
                                     (0, 2): (pbA, 64), (1, 0): (pbA, 96),
                                     (1, 1): (pbB, 0), (1, 2): (pbB, 32)}
                            psb_r = psb.rearrange("p (a b w) -> p a b w", a=8, b=8)
                            psb_s = psb.rearrange("p (k d) -> p k d", d=8)
                            for hh in range(2):
                                for ti in range(3):
                                    pbt, r0 = slots[(hh, ti)]
                                    for dlt in range(8):
                                        if ti == 0:    # d: stream (b,w)
                                            rhs = psb[:, dlt * 64:(dlt + 1) * 64]
                                        elif ti == 1:  # w: stream (a,b)
                                            rhs = psb_s[:, :, dlt]
                                        else:          # h: stream (a,w)
                                            rhs = psb_r[:, :, dlt, :]
                                        tsel = (0, 2, 1)[ti]
                                        sc = (tsel * 8 + dlt) * 16 + hh * 8
                                        nc.tensor.matmul(
                                            pbt[r0:r0 + 8, dlt * 64:(dlt + 1) * 64],
                                            shsel[:, sc:sc + 8], rhs,
                                            start=(dlt == 0), stop=(dlt == 7),
                                            tile_position=(0, r0),
                                            skip_group_check=True)
                            # evacuate: d rows go straight to qhat (both
                            # ends 32-aligned); w+h go via one staging tile
                            # (permuted cols) then one identity DMA
                            for hh in range(2):
                                h = 2 * (m - 12) + hh
                                pbt_d, r_d = slots[(hh, 0)]
                                pbt_w, r_w = slots[(hh, 1)]
                                pbt_h, r_h = slots[(hh, 2)]
                                nc.vector.tensor_copy(qhat[h][0:8, cols],
                                                      pbt_d[r_d:r_d + 8, :])
                                stg = psb_pool.tile([8, 1024], BF16, tag="stg",
                                                    name=f"stg{hh}_{m}_{ch}")
                                # w: src (dlt, a, b) -> dst cols a*64+b*8+dlt
                                src_w = pbt_w[r_w:r_w + 8, :].rearrange(
                                    "p (d a b) -> p d a b", d=8, a=8)
                                dst_w = stg[:, 0:512].rearrange(
                                    "p (a b d) -> p d a b", a=8, b=8)
                                nc.vector.tensor_copy(dst_w, src_w)
                                # h: src (dlt, a, w) -> dst cols a*64+dlt*8+w
                                src_h = pbt_h[r_h:r_h + 8, :].rearrange(
                                    "p (d a w) -> p d a w", d=8, a=8)
                                dst_h = stg[:, 512:1024].rearrange(
                                    "p (a d w) -> p d a w", a=8, d=8)
                                if hh == 0:
                                    nc.vector.tensor_copy(dst_h, src_h)
                                else:
                                    nc.scalar.copy(dst_h, src_h)
                                # one DMA: stg halves -> qhat rows 8-15 / 16-23
                                qeng = nc.sync if hh == 0 else nc.gpsimd
                                qeng.dma_start(out=qhat[h][8:16, cols],
                                               in_=stg[:, 0:512])
                                qeng.dma_start(out=qhat[h][16:24, cols],
                                               in_=stg[:, 512:1024])
                # attention for this head pair
                for h in (2 * hp, 2 * hp + 1):
                    for wi in range(NWIN):
                        qcols = bass.ds(wi * 512, 512)
                        ptp = []
                        for pp in range(2):
                            ptile = pt_pool.tile([128, 1024], F8, tag="pt")
                            for kk in range(2):
                                kt = pp * 2 + kk
                                ps = s_ps.tile([128, 512], F32, tag="sps")
                                nc.tensor.matmul(
                                    ps, khat[h][:, wi * 512 + kt * 128: wi * 512 + (kt + 1) * 128],
                                    qhat[h][:, qcols], start=True, stop=True)
                                nc.scalar.activation(ptile[:, kk * 512:(kk + 1) * 512],
                                                     ps, Exp)
                            ptp.append(ptile)
                        pc = c_ps.tile([65, 512], F32, tag="cps")
                        for pp in range(2):
                            vv = vhat[wi * 2 + pp].rearrange(
                                "p (ko hc) -> p ko hc", ko=2, hc=784)
                            nc.tensor.matmul(
                                pc, vv[:, :, h * 65:h * 65 + 65],
                                ptp[pp].rearrange("p (ko n) -> p ko n", ko=2),
                                start=(pp == 0), stop=(pp == 1),
                                perf_mode=DR)
                        rec = dn_pool.tile([1, 512], BF16, tag="rec")
                        nc.vector.reciprocal(rec, pc[64:65, :])
                        pb64 = r_ps.tile([64, 512], F32, tag="rps")
                        nc.tensor.matmul(pb64, ones_r64, rec, start=True, stop=True)
                        bb = dn_pool.tile([64, 512], BF16, tag="bb")
                        nc.scalar.copy(bb, pb64)
                        cdst = ctxv[h // 4][(h % 2) * 64:(h % 2) * 64 + 64,
                                            (h // 2) % 2, qcols]
                        nc.vector.scalar_tensor_tensor(
                            out=cdst, in0=pc[0:64, :], scalar=c16[0:64, :],
                            in1=bb, op0=mybir.AluOpType.mult,
                            op1=mybir.AluOpType.mult)

        xh_cm.__exit__(None, None, None)
        vh_cm.__exit__(None, None, None)
        kh_cm.__exit__(None, None, None)
        qh_cm.__exit__(None, None, None)
        wq_cm.__exit__(None, None, None)

        # MLP weights: loaded now (space freed by attention pools); the
        # transfers overlap proj + LN2
        w1_cm = tc.tile_pool(name="w1", bufs=KT)
        w1_pool = w1_cm.__enter__()
        w1s = [w1_pool.tile([128, 3072], BF16, tag="w1", name=f"w1_{i}") for i in range(KT)]
        w2_cm = tc.tile_pool(name="w2", bufs=24)
        w2_pool = w2_cm.__enter__()
        w2s = [w2_pool.tile([128, 768], BF16, tag="w2", name=f"w2_{i}") for i in range(24)]
        for k in range(KT):
            nc.sync.dma_start(out=w1s[k], in_=d["w1"][k * 128:(k + 1) * 128, :])
        for k in range(24):
            nc.sync.dma_start(out=w2s[k], in_=d["w2"][k * 128:(k + 1) * 128, :])

        # ---- proj + residual -> x2 (fp32) ----
        x2_cm = tc.tile_pool(name="x2", bufs=KT)
        x2_pool = x2_cm.__enter__()
        x2 = [x2_pool.tile([128, NTOK], BF16, tag="x2", name=f"x2_{i}") for i in range(KT)]
        with ExitStack() as ph:
            p_ps = ph.enter_context(tc.tile_pool(name="p_ps", bufs=4, space="PSUM"))
            for ch in range(2):
                cols = bass.ds(ch * 512, 512)
                for m in range(KT):
                    pp = p_ps.tile([128, 512], F32, tag="pps")
                    for g in range(3):
                        nc.tensor.matmul(pp, wpv[:, g, :, m * 128:(m + 1) * 128],
                                         ctxv[g][:, :, cols],
                                         start=(g == 0), stop=(g == 2),
                                         perf_mode=DR)
                    nc.vector.scalar_tensor_tensor(
                        out=x2[m][:, cols], in0=pp, scalar=inv1024[:, 0:1],
                        in1=xs[m][:, cols], op0=mybir.AluOpType.mult, op1=ADD)

        # ---- LN2 -> mh (bf16) ----
        mh_cm = tc.tile_pool(name="mh", bufs=KT)
        mh_pool = mh_cm.__enter__()
        mh = [mh_pool.tile([128, NTOK], BF16, tag="mh", name=f"mh{i}") for i in range(KT)]

        def ln2_out(k, ch, sb, bc_rs):
            nc.gpsimd.tensor_mul(mh[k][:, bass.ds(ch * 512, 512)], sb, bc_rs)
        _emit_ln(nc, tc, x2, ln2_out, consts)

        # ---- fc1 + gelu -> h1 (bf16) ----
        h1_cm = tc.tile_pool(name="h1", bufs=24)
        h1_pool = h1_cm.__enter__()
        h1 = [h1_pool.tile([128, NTOK], BF16, tag="h1", name=f"h1_{i}") for i in range(24)]
        with ExitStack() as ph:
            f1_ps = ph.enter_context(tc.tile_pool(name="f1_ps", bufs=6, space="PSUM"))
            gtmp = ph.enter_context(tc.tile_pool(name="gtmp", bufs=4)) if SIM_GELU else None
            for m in range(24):
                for ch in range(2):
                    pf = f1_ps.tile([128, 512], F32, tag="f1ps")
                    for g in range(KT):
                        nc.tensor.matmul(
                            pf, w1s[g][:, m * 128:(m + 1) * 128],
                            mh[g][:, ch * 512:(ch + 1) * 512],
                            start=(g == 0), stop=(g == KT - 1))
                    h1dst = h1[m][:, ch * 512:(ch + 1) * 512]
                    if SIM_GELU:
                        xb = gtmp.tile([128, 512], BF16, tag="xb")
                        nc.scalar.activation(xb, pf, Ident,
                                             bias=b1[:, m:m + 1], scale=1.0)
                        sg = gtmp.tile([128, 512], BF16, tag="sg")
                        nc.scalar.activation(sg, xb, Sigmoid, scale=1.702)
                        nc.vector.tensor_mul(h1dst, xb, sg)
                    else:
                        nc.scalar.activation(h1dst, pf, Gelu,
                                             bias=b1[:, m:m + 1], scale=1.0)

        # ---- fc2 + residual -> out ----
        with ExitStack() as ph:
            f2_ps = ph.enter_context(tc.tile_pool(name="f2_ps", bufs=4, space="PSUM"))
            o_pool = ph.enter_context(tc.tile_pool(name="outT", bufs=2))
            for m in range(KT):
                ot = o_pool.tile([128, NTOK], F32, tag="ot")
                for ch in range(2):
                    pf = f2_ps.tile([128, 512], F32, tag="f2ps")
                    for g in range(24):
                        nc.tensor.matmul(
                            pf, w2s[g][:, m * 128:(m + 1) * 128],
                            h1[g][:, ch * 512:(ch + 1) * 512],
                            start=(g == 0), stop=(g == 23))
                    nc.vector.scalar_tensor_tensor(
                        out=ot[:, ch * 512:(ch + 1) * 512],
                        in0=pf, scalar=b2[:, m:m + 1],
                        in1=x2[m][:, ch * 512:(ch + 1) * 512],
                        op0=ADD, op1=ADD)
                nc.sync.dma_start(out=d["outT"][m * 128:(m + 1) * 128, :], in_=ot)

        h1_cm.__exit__(None, None, None)
        mh_cm.__exit__(None, None, None)
        x2_cm.__exit__(None, None, None)
        w2_cm.__exit__(None, None, None)
        w1_cm.__exit__(None, None, None)
        ctx_cm.__exit__(None, None, None)


def _build(loop_n=None):
    nc = bacc.Bacc("TRN2", target_bir_lowering=False, debug=False, num_devices=8)
    dd = {}

    dd["xT"] = nc.dram_tensor("xT", [DIM, NTOK], BF16, kind="ExternalInput").ap()
    dd["baug"] = nc.dram_tensor("baug", [1, 2304], BF16, kind="ExternalInput").ap()
    dd["bp"] = nc.dram_tensor("bp", [128, 6], F32, kind="ExternalInput").ap()
    dd["b1"] = nc.dram_tensor("b1", [128, 24], F32, kind="ExternalInput").ap()
    dd["b2"] = nc.dram_tensor("b2", [128, 6], F32, kind="ExternalInput").ap()
    dd["waug"] = nc.dram_tensor("waug", [DIM, 2304], BF16, kind="ExternalInput").ap()
    dd["wv"] = nc.dram_tensor("wv", [DIM, DIM], BF16, kind="ExternalInput").ap()
    dd["wp"] = nc.dram_tensor("wp", [128, 3 * 2 * 768], F8, kind="ExternalInput").ap()
    dd["w1"] = nc.dram_tensor("w1", [DIM, 3072], BF16, kind="ExternalInput").ap()
    dd["w2"] = nc.dram_tensor("w2", [3072, DIM], BF16, kind="ExternalInput").ap()
    dd["bv"] = nc.dram_tensor("bv", [1, DIM], BF16, kind="ExternalInput").ap()
    dd["e2"] = nc.dram_tensor("e2", [64, NTOK], BF16, kind="ExternalInput").ap()
    dd["shsel"] = nc.dram_tensor("shsel", [128, 384], BF16, kind="ExternalInput").ap()
    dd["outT"] = nc.dram_tensor("outT", [DIM, NTOK], F32, kind="ExternalOutput").ap()

    with tile.TileContext(nc) as tc:
        if loop_n is None:
            _emit(nc, tc, dd)
        else:
            with tc.For_i(0, loop_n, 1):
                _emit(nc, tc, dd)
    nc.compile()
    return nc


# ---------------------------------------------------------------------------
# host side
# ---------------------------------------------------------------------------

def _col_tiles(b):
    """(n*128,) bias -> (128, n) column-tile layout."""
    n = b.shape[0] // 128
    return np.ascontiguousarray(b.reshape(n, 128).T)


def _to_f8(a):
    return np.asarray(a, np.float32).astype(ml_dtypes.float8_e4m3)


def prep_weights(inputs):
    g = {k: np.asarray(v, np.float32) for k, v in inputs.items()}
    qkv_w, qkv_b = g["qkv_w"], g["qkv_b"]
    ln1_w, ln1_b = g["ln1_w"], g["ln1_b"]
    Wf = qkv_w * ln1_w[None, :]
    bf = qkv_b + qkv_w @ ln1_b
    Wq, bq = Wf[0:768], bf[0:768]
    Wk, bk = Wf[768:1536], bf[768:1536]
    Wv, bv = Wf[1536:2304], bf[1536:2304]
    rel = (g["rel_pos_d"], g["rel_pos_h"], g["rel_pos_w"])
    W_aug = np.zeros((2304, 768), np.float32)
    b_aug = np.zeros((2304,), np.float32)
    W_aug[0:768] = Wq * SCALE
    b_aug[0:768] = bq * SCALE
    W_aug[768:1536] = Wk
    b_aug[768:1536] = bk
    for h in range(NH):
        Wq_h, bq_h = Wq[h * 64:(h + 1) * 64], bq[h * 64:(h + 1) * 64]
        for ti in range(3):
            T = rel[ti][::-1]
            rows = 1536 + h * 64 + ti * 15
            W_aug[rows:rows + 15] = T @ Wq_h
            b_aug[rows:rows + 15] = T @ bq_h
    m = np.arange(512)
    # khat one-hot rows: 0-7 e_d, 8-15 e_w, 16-23 e_h, zeros elsewhere
    # (matches qhat bias rows: d 0-7, w 8-15, h 16-23)
    E = np.zeros((64, 512), np.float32)
    E[0 + m // 64, m] = 1.0
    E[8 + m % 8, m] = 1.0
    E[16 + (m // 8) % 8, m] = 1.0
    # shift-selector one-hots: for (ti, dlt), 16 output rows (j + 8*hh)
    # selecting psb row hh*64 + ti*15 + 7 - dlt + j
    shsel = np.zeros((128, 3 * 8 * 16), np.float32)
    for ti in range(3):
        for dlt in range(8):
            for hh in range(2):
                for j in range(8):
                    r = hh * 64 + ti * 15 + 7 - dlt + j
                    shsel[r, (ti * 8 + dlt) * 16 + hh * 8 + j] = 1.0
    return {
        "shsel": np.ascontiguousarray(shsel).astype(ml_dtypes.bfloat16),
        "waug": np.ascontiguousarray(W_aug.T).astype(ml_dtypes.bfloat16),
        "baug": np.ascontiguousarray(b_aug[None, :]).astype(ml_dtypes.bfloat16),
        "wv": np.ascontiguousarray(Wv.T).astype(ml_dtypes.bfloat16),
        "bv": np.ascontiguousarray(bv[None, :]).astype(ml_dtypes.bfloat16),
        "wp": np.ascontiguousarray(
            (g["proj_w"].T * 64.0).reshape(3, 2, 128, 768)
            .transpose(2, 0, 1, 3).reshape(128, -1)).astype(ml_dtypes.float8_e4m3),
        "bp": _col_tiles(g["proj_b"]),
        "w1": np.ascontiguousarray((g["fc1_w"] * g["ln2_w"][None, :]).T).astype(ml_dtypes.bfloat16),
        "b1": _col_tiles(g["fc1_b"] + g["fc1_w"] @ g["ln2_b"]),
        "w2": np.ascontiguousarray(g["fc2_w"].T).astype(ml_dtypes.bfloat16),
        "b2": _col_tiles(g["fc2_b"]),
        "e2": np.ascontiguousarray(np.concatenate([E, E], axis=1)).astype(ml_dtypes.bfloat16),
    }


def shard_x(x):
    """x (B,D,H,W,C) -> list of 8 per-core (768, 1024) bf16 C-layout arrays."""
    B, D, H, W, C = x.shape
    win = x.reshape(B, D // WS, WS, H // WS, WS, W // WS, WS, C)
    win = win.transpose(0, 1, 3, 5, 2, 4, 6, 7).reshape(-1, WS ** 3, C)
    return [np.ascontiguousarray(win[2 * c:2 * c + 2].reshape(NTOK, C).T).astype(ml_dtypes.bfloat16)
            for c in range(8)]


def unshard_out(outs, shape):
    B, D, H, W, C = shape
    full = np.concatenate([o.T for o in outs], axis=0).reshape(16, WS ** 3, C)
    x = full.reshape(B, D // WS, H // WS, W // WS, WS, WS, WS, C)
    x = x.transpose(0, 1, 4, 2, 5, 3, 6, 7).reshape(B, D, H, W, C)
    return np.ascontiguousarray(x)


_STATE = {}


def _make_runner(nc):
    """Wrap a compiled Bass program in a persistent jitted SPMD callable."""
    import jax
    from jax.sharding import Mesh, PartitionSpec
    from jax.experimental.shard_map import shard_map
    from concourse import bass2jax

    bass2jax.install_neuronx_cc_hook()

    n_cores = 8
    partition_name = nc.partition_id_tensor.name if nc.partition_id_tensor else None
    in_names, out_names, out_avals, zero_outs = [], [], [], []
    for alloc in nc.m.functions[0].allocations:
        if not isinstance(alloc, mybir.MemoryLocationSet):
            continue
        name = alloc.memorylocations[0].name
        if alloc.kind == "ExternalInput":
            if name != partition_name:
                in_names.append(name)
        elif alloc.kind == "ExternalOutput":
            out_names.append(name)
            shape = tuple(alloc.tensor_shape)
            dtype = mybir.dt.np(alloc.dtype)
            out_avals.append(jax.core.ShapedArray(shape, dtype))
            zero_outs.append(np.zeros(shape, dtype))
    n_params = len(in_names)
    all_in_names = in_names + out_names
    if partition_name is not None:
        all_in_names = all_in_names + [partition_name]

    def _body(*args):
        operands = list(args)
        if partition_name is not None:
            operands.append(bass2jax.partition_id_tensor())
        outs = bass2jax._bass_exec_p.bind(
            *operands,
            out_avals=tuple(out_avals),
            in_names=tuple(all_in_names),
            out_names=tuple(out_names),
            lowering_input_output_aliases=(),
            sim_require_finite=True,
            sim_require_nnan=True,
            nc=nc,
        )
        return tuple(outs)

    devices = jax.devices()[:n_cores]
    mesh = Mesh(np.asarray(devices), ("core",))
    donate = tuple(range(n_params, n_params + len(out_names)))
    sharded = jax.jit(
        shard_map(_body, mesh=mesh,
                  in_specs=(PartitionSpec("core"),) * (n_params + len(out_names)),
                  out_specs=(PartitionSpec("core"),) * len(out_names)),
        donate_argnums=donate, keep_unused=True)

    def run(in_maps):
        per_core = [[np.asarray(m[nm]) for nm in in_names] for m in in_maps]
        concat_in = [np.concatenate([per_core[c][i] for c in range(n_cores)], axis=0)
                     for i in range(n_params)]
        concat_zero = [np.zeros((n_cores * z.shape[0], *z.shape[1:]), z.dtype)
                       for z in zero_outs]
        out_arrs = sharded(*concat_in, *concat_zero)
        return [
            {nm: np.asarray(out_arrs[i]).reshape(n_cores, *out_avals[i].shape)[c]
             for i, nm in enumerate(out_names)}
            for c in range(n_cores)
        ]

    return run, dict(sharded=sharded, body=_body, in_names=in_names,
                     out_names=out_names, out_avals=out_avals,
                     zero_outs=zero_outs, mesh=mesh, n_params=n_params)


def _get_runner():
    if "run" not in _STATE:
        run, internals = _make_runner(_build())
        _STATE["run"] = run
        _STATE["internals"] = internals
    return _STATE["run"]


def kernel(**inputs):
    x = np.asarray(inputs["x"], np.float32)
    w = prep_weights(inputs)
    shards = shard_x(x)
    in_maps = [dict(w, xT=shards[c]) for c in range(8)]
    run = _get_runner()
    results = run(in_maps)
    outs = [results[c]["outT"] for c in range(8)]
    return unshard_out(outs, x.shape)



# revision 39
# speedup vs baseline: 1.1310x; 1.1310x over previous
"""Trainium2 Bass kernel for a 3D windowed-attention transformer block.

Data-parallel over the 16 attention windows (2 windows/core x 8 cores).
Device tensors live in C-layout [channels(partitions), tokens(free)].

Key design points vs the earlier baseline:
- Weight DMAs are issued ahead of use into non-aliasing pools (attention
  weights at t=0, MLP weights as soon as attention SBUF frees), so the
  PE never stalls on SBUF reuse.
- x ships as bf16; LayerNorm runs with PE stats/broadcasts, DVE subs and
  GPSIMD (SBUF-only) multiplies.
- The decomposed rel-pos bias scatter uses per-offset shift-selector
  matmuls (N=64, M=8) into 32-aligned PSUM slots, evacuated by aligned
  copies (d goes straight to qhat; w/h via one bf16 staging tile and two
  identity SBUF-to-SBUF DMAs per head). This replaces the baseline's 192
  gpsimd DMAs + 96 N=512 selector matmuls + 384 small DVE copies.
- qkv bias is accumulated in PSUM via a K=1 rank-1 matmul; the ACT
  engine mostly runs exp/gelu plus a few evacuations.
- The attention PV contraction and the output projection run in
  fp8-e4m3 DoubleRow (exp output bounded well inside e4m3 range; proj
  weights prescaled x64 and ctx x16 to escape the e4m3 subnormal zone,
  compensated by a 1/1024 scale folded into the residual add). vhat
  ko-stride padded to 784 for the 16B DoubleRow alignment rule. The MLP
  stays bf16: fp8 there costs ~4% relative error on the MLP branch
  (~2e-2 end-to-end), over the gate.
- Hardware legality notes baked in: GPSIMD cannot touch PSUM; compute
  ops need one PSUM operand max and 32-aligned partition bases; DMA APs
  cross partitions only via their first dim; matmul PSUM base must equal
  tile_position[1] (explicit for the 96-base slot).
"""

import ml_dtypes
import numpy as np

import concourse.bass as bass
import concourse.mybir as mybir
import concourse.tile as tile
from concourse import bacc

DIM = 768
NH = 12
HD = 64
WS = 8
NTOK = 1024          # tokens per core (2 windows x 512)
NWIN = 2
KT = DIM // 128      # 6 k-tiles over channels
SCALE = HD ** -0.5
EPS = 1e-5
F32 = mybir.dt.float32
F32R = mybir.dt.float32r
BF16 = mybir.dt.bfloat16
F8 = mybir.dt.float8e4

Ident = mybir.ActivationFunctionType.Identity
Exp = mybir.ActivationFunctionType.Exp
Gelu = mybir.ActivationFunctionType.Gelu
Sigmoid = mybir.ActivationFunctionType.Sigmoid
Sqrt = mybir.ActivationFunctionType.Sqrt
Rsqrt = mybir.ActivationFunctionType.Rsqrt
ADD = mybir.AluOpType.add
DR = mybir.MatmulPerfMode.DoubleRow

# CoreSim has no Gelu; set True (sim only) to use x*sigmoid(1.702x) instead
SIM_GELU = False
# bisection toggles
PIPELINE = True     # software-pipeline scatter(hp) ahead of attention(hp-1)
TWO_ACC = True      # ch-inner two-accumulator matmul order (LDW sharing)
BIAS_EVAC = True    # biases via evac ops (0: plain copies, no bias - debug only)
FAST_RECIP = True   # reciprocal_approx_fast + gpsimd broadcast in softmax
LN_GPS = True       # gpsimd broadcasts + fast recip in layernorm


# ---------------------------------------------------------------------------
# device program
# ---------------------------------------------------------------------------

def _emit_ln(nc, tc, xs, write_out, consts):
    """Standardize along channels (partition dim): out = (x - mean)*rstd.

    xs: 6 input tiles [128, 1024] (bf16 or fp32).
    write_out(k, ch, sb_ap, bc_r_ap): final multiply destination hook.
    Broadcasts run on gpsimd (partition_broadcast); 1/sqrt via the fast
    DVE Newton-Raphson reciprocal (no PE broadcast matmuls, no slow
    InstReciprocal).
    """
    from contextlib import ExitStack
    with ExitStack() as ctx:
        stat_ps = ctx.enter_context(tc.tile_pool(name="ln_stat_ps", bufs=2, space="PSUM"))
        sq_pool = ctx.enter_context(tc.tile_pool(name="ln_sq", bufs=2))
        st_pool = ctx.enter_context(tc.tile_pool(name="ln_st", bufs=2))
        sb_pool = ctx.enter_context(tc.tile_pool(name="ln_sb", bufs=4))
        if not LN_GPS:
            consts["bc_ps"] = ctx.enter_context(
                tc.tile_pool(name="ln_bc_ps", bufs=2, space="PSUM"))
        for ch in range(2):
            cols = bass.ds(ch * 512, 512)
            ps_s = stat_ps.tile([1, 512], F32, tag="ps_s")
            ps_q = stat_ps.tile([1, 512], F32, tag="ps_q")
            for k in range(KT):
                sq = sq_pool.tile([128, 512], BF16, tag="sq")
                # squares on ACT (Square): gpsimd only does the broadcasts,
                # so the stats matmuls aren't gated on a busy gpsimd queue
                nc.scalar.activation(sq, xs[k][:, cols],
                                     mybir.ActivationFunctionType.Square)
                nc.tensor.matmul(ps_s, consts["ones_col"], xs[k][:, cols],
                                 start=(k == 0), stop=(k == KT - 1))
                nc.tensor.matmul(ps_q, consts["ones_col"], sq,
                                 start=(k == 0), stop=(k == KT - 1))
            mean = st_pool.tile([1, 512], BF16, tag="mean")
            ex2 = st_pool.tile([1, 512], F32, tag="ex2")
            nc.scalar.mul(mean, ps_s, 1.0 / DIM)
            nc.scalar.mul(ex2, ps_q, 1.0 / DIM)
            mm = st_pool.tile([1, 512], F32, tag="mm")
            nc.vector.tensor_mul(mm, mean, mean)
            var = st_pool.tile([1, 512], F32, tag="var")
            nc.vector.tensor_sub(var, ex2, mm)
            rstd = st_pool.tile([1, 512], F32, tag="rstd")
            nc.scalar.activation(rstd, var, Sqrt,
                                 bias=consts["eps"][0:1, 0:1], scale=1.0)
            rstd_b = st_pool.tile([1, 512], BF16, tag="rstd_b")
            if LN_GPS:
                rstd_f = st_pool.tile([1, 512], F32, tag="rstd_f")
                nc.vector.reciprocal_approx_fast(rstd_f, rstd)
                nc.scalar.copy(rstd_b, rstd_f)
                bc_m = sb_pool.tile([128, 512], BF16, tag="bc_m")
                bc_rs = sb_pool.tile([128, 512], BF16, tag="bc_rs")
                nc.gpsimd.partition_broadcast(bc_m, mean)
                nc.gpsimd.partition_broadcast(bc_rs, rstd_b)
            else:
                nc.vector.reciprocal(rstd_b, rstd)
                bc_mp = consts["bc_ps"].tile([128, 512], F32, tag="bc_m")
                bc_rp = consts["bc_ps"].tile([128, 512], F32, tag="bc_r")
                nc.tensor.matmul(bc_mp, consts["ones_row"], mean, start=True, stop=True)
                nc.tensor.matmul(bc_rp, consts["ones_row"], rstd_b, start=True, stop=True)
                bc_m = bc_mp
                bc_rs = sb_pool.tile([128, 512], BF16, tag="bc_rs")
                nc.scalar.copy(bc_rs, bc_rp)
            for k in range(KT):
                sb = sb_pool.tile([128, 512], BF16, tag="sb")
                seng = nc.vector if k % 2 == 0 else nc.gpsimd
                seng.tensor_sub(sb, xs[k][:, cols], bc_m)
                write_out(k, ch, sb, bc_rs)


def _emit(nc, tc, d):
    """Emit the whole per-core program. d: dict of DRAM APs."""
    from contextlib import ExitStack

    with ExitStack() as top:
        top.enter_context(nc.allow_low_precision(
            reason="bf16/fp8 matmul operands are intentional; accumulation stays fp32"))

        # ---- persistent constants & weights; DMAs all start here ----
        consts_pool = top.enter_context(tc.tile_pool(name="consts", bufs=1))
        ones_col = consts_pool.tile([128, 1], BF16, tag="ones_col")
        nc.vector.memset(ones_col, 1.0)

        ones_row = consts_pool.tile([1, 128], BF16, tag="ones_row")
        nc.vector.memset(ones_row, 1.0)
        eps = consts_pool.tile([1, 1], F32, tag="eps")
        nc.vector.memset(eps, EPS)
        consts = dict(ones_col=ones_col, ones_row=ones_row, eps=eps)

        # x input first (LN1 needs it), then weights in order of use
        xs_pool = top.enter_context(tc.tile_pool(name="xs", bufs=KT))
        xs = [xs_pool.tile([128, NTOK], BF16, tag="xs", name=f"xs{i}") for i in range(KT)]
        for k in range(KT):
            nc.sync.dma_start(out=xs[k], in_=d["xT"][k * 128:(k + 1) * 128, :])

        e2_sb = consts_pool.tile([64, NTOK], BF16, tag="e2")
        nc.sync.dma_start(out=e2_sb, in_=d["e2"])
        shsel = consts_pool.tile([128, 384], BF16, tag="shsel")
        nc.sync.dma_start(out=shsel, in_=d["shsel"])
        bcol = consts_pool.tile([128, 18], F32, tag="bcol")
        nc.sync.dma_start(out=bcol, in_=d["bcol"])
        bv = consts_pool.tile([1, 768], BF16, tag="bv")
        nc.sync.dma_start(out=bv, in_=d["bv"])
        b1 = consts_pool.tile([128, 24], F32, tag="b1")
        nc.sync.dma_start(out=b1, in_=d["b1"])
        b2 = consts_pool.tile([128, 6], F32, tag="b2")
        nc.sync.dma_start(out=b2, in_=d["b2"])

        # later-phase weights: pools persistent (top), DMAs issued after the
        # attention-phase weights below (queue order = call order)
        wp_pool = top.enter_context(tc.tile_pool(name="wp", bufs=1))
        wpt = wp_pool.tile([128, 3 * 2 * 768], F8, tag="wpt")
        wpv = wpt.rearrange("p (g ko m) -> p g ko m", g=3, ko=2)

        # ---- scoped activation tensors (manual LIFO: enter ctx, then
        # attention-phase pools; exit attention pools in reverse, ctx after
        # proj) ----
        ctx_cm = tc.tile_pool(name="ctxT", bufs=3)
        ctx_pool = ctx_cm.__enter__()
        ctxT3 = [ctx_pool.tile([128, 2 * NTOK], F8, tag="ctxT", name=f"ctxT{i}") for i in range(3)]
        ctxv = [t.rearrange("p (ko n) -> p ko n", ko=2) for t in ctxT3]

        wq_cm = tc.tile_pool(name="wqfull", bufs=3)
        qh_cm = tc.tile_pool(name="qhat", bufs=NH)
        kh_cm = tc.tile_pool(name="khat", bufs=NH)
        vh_cm = tc.tile_pool(name="vhat", bufs=4)
        xh_cm = tc.tile_pool(name="xhat", bufs=3)
        wq_pool = wq_cm.__enter__()
        qh_pool = qh_cm.__enter__()
        kh_pool = kh_cm.__enter__()
        vh_pool = vh_cm.__enter__()
        xh_pool = xh_cm.__enter__()
        # vhat head slots are 128 wide: col 0 = ones (softmax denominator ->
        # PSUM partition 0, so the custom-DVE reciprocal needs no partition
        # shift), cols 64-127 = v channels (ctx lands at aligned PSUM base 64;
        # >32-partition accesses must start at base 0 or 64)
        VSLOT = 128
        VKO = NH * VSLOT  # 1536 per ko half
        # qkv/v/fc1 run in fp8-e4m3 DoubleRow: channel c of DR tile j sits
        # at [p=c%128, ko=(c%256)//128]; weights prescaled (q/P x256, k x32,
        # v/fc1 x32), descaled by exact powers of two at evacuation
        wqs = [wq_pool.tile([128, 2 * 2304], F8, tag="wqf", name=f"wqf{i}") for i in range(3)]
        wqv = [t.rearrange("p (ko m) -> p ko m", ko=2) for t in wqs]
        qhat = [qh_pool.tile([128, NTOK], BF16, tag="qhat", name=f"qhat{i}") for i in range(NH)]
        khat = [kh_pool.tile([128, NTOK], BF16, tag="khat", name=f"khat{i}") for i in range(NH)]
        vhat = [vh_pool.tile([128, 2 * VKO], F8, tag="vhat", name=f"vhat{i}") for i in range(4)]
        xhat = [xh_pool.tile([128, 2 * NTOK], F8, tag="xh", name=f"xh{i}") for i in range(3)]
        xhv = [t.rearrange("p (ko n) -> p ko n", ko=2) for t in xhat]

        # v weights: scoped pool (freed right after the v projection);
        # DMA goes on the scalar queue so the sync queue keeps priority order
        wv_cm = tc.tile_pool(name="wv", bufs=KT)
        wv_pool = wv_cm.__enter__()
        wvs = [wv_pool.tile([128, 2 * 768], F8, tag="wv", name=f"wv{i}") for i in range(3)]
        wvv = [t.rearrange("p (ko n) -> p ko n", ko=2) for t in wvs]
        for j in range(3):
            nc.scalar.dma_start(out=wvs[j], in_=d["wv"][:, j * 1536:(j + 1) * 1536])

        # weight DMAs on sync, in order of first use
        for j in range(3):
            nc.sync.dma_start(out=wqs[j], in_=d["waug"][:, j * 4608:(j + 1) * 4608])
        nc.sync.dma_start(out=wpt, in_=d["wp"])

        # khat one-hot key-position rows; qhat memset (rows 16-31/40-63 must be
        # finite; bias rows and q rows get overwritten)
        for h in range(NH):
            nc.sync.dma_start(out=khat[h][0:64, :], in_=d["e2"])
            nc.gpsimd.memset(qhat[h], 0.0)
        for t in range(4):
            nc.vector.memset(vhat[t], 0.0)
            vv = vhat[t].rearrange("p (ko h c) -> p ko h c", ko=2, c=VSLOT)
            nc.vector.memset(vv[:, :, :, 0:1], 1.0)

        # ---- LN1 -> xhat (fp8 DR pairs) ----
        def ln1_out(k, ch, sb, bc_rs):
            eng = nc.gpsimd if k % 2 == 0 else nc.vector
            eng.tensor_mul(xhv[k // 2][:, k % 2, bass.ds(ch * 512, 512)], sb, bc_rs)
        _emit_ln(nc, tc, xs, ln1_out, consts)

        # ---- v projection -> vhat (fp8 pairs for DoubleRow PV) ----
        # nch-inner 2-accumulator order: consecutive matmuls share the
        # stationary xhat chunk (LDWEIGHTS reuse)
        with ExitStack() as vph:
            v_ps = vph.enter_context(tc.tile_pool(name="v_ps", bufs=4, space="PSUM"))
            for t in range(8):
                pvs = [v_ps.tile([128, 384], F32, tag="vps",
                                 name=f"vps_{t}_{nch}") for nch in range(2)]
                for j in range(3):
                    for nch in range(2):
                        nc.tensor.matmul(pvs[nch], xhv[j][:, :, t * 128:(t + 1) * 128],
                                         wvv[j][:, :, nch * 384:(nch + 1) * 384],
                                         start=(j == 0), stop=False, perf_mode=DR)
                for nch in range(2):
                    nc.tensor.matmul(pvs[nch], ones_row,
                                     bv[0:1, nch * 384:(nch + 1) * 384],
                                     start=False, stop=True)
                # psum holds 32*v; evac scale 1/2 stores vhat = 16*v (the
                # x16 fp8 pre-scale previously applied in the ctx STT)
                for nch in range(2):
                    vv = vhat[t // 2][:, (t % 2) * VKO + nch * 6 * VSLOT:
                                      (t % 2) * VKO + (nch + 1) * 6 * VSLOT].rearrange(
                        "p (h c) -> p h c", c=VSLOT)
                    dst = vv[:, :, 64:128]
                    src = pvs[nch].rearrange("p (h c) -> p h c", c=64)
                    if nch == 0:
                        nc.vector.tensor_scalar_mul(dst, src, 0.5)
                    else:
                        nc.scalar.mul(dst, src, 0.5)
        wv_cm.__exit__(None, None, None)

        # ---- qkv + rel-pos scatter + attention, software-pipelined ----
        # round hp emits qkv+scatter for head-pair hp, THEN attention for
        # head-pair hp-1: the scatter's staging DMAs get a full attention
        # round to land before the scores that consume them.
        with ExitStack() as ph:
            qkv_ps = ph.enter_context(tc.tile_pool(
                name="qkv_ps", bufs=(3 if FAST_RECIP else 2), space="PSUM"))
            pb_ps = ph.enter_context(tc.tile_pool(name="pb_ps", bufs=1, space="PSUM"))
            psb_pool = ph.enter_context(tc.tile_pool(name="psb", bufs=2))
            s_ps = ph.enter_context(tc.tile_pool(name="s_ps", bufs=2, space="PSUM"))
            c_ps = ph.enter_context(tc.tile_pool(name="c_ps", bufs=1, space="PSUM"))
            pt_pool = ph.enter_context(tc.tile_pool(name="ptile", bufs=2))
            dn_pool = ph.enter_context(tc.tile_pool(name="dn", bufs=2))
            aux = {}
            if not FAST_RECIP:
                aux["r_ps"] = ph.enter_context(
                    tc.tile_pool(name="r_ps", bufs=1, space="PSUM"))
                ones_r64 = consts_pool.tile([1, 64], BF16, tag="ones_r64")
                nc.vector.memset(ones_r64, 1.0)
                aux["ones_r64"] = ones_r64

            def emit_qkv_scatter(hp):
                # m order: P rows first so the scatter+DMA chain starts ASAP
                for m in (12 + hp, hp, 6 + hp):
                    pts = [qkv_ps.tile([128, 512], F32, tag="qkvps",
                                       name=f"qkvps_{m}_{ch}") for ch in range(2)]
                    # ch-inner: both accumulators share each stationary wq chunk
                    for j in range(3):
                        for ch in range(2):
                            nc.tensor.matmul(
                                pts[ch], wqv[j][:, :, m * 128:(m + 1) * 128],
                                xhv[j][:, :, bass.ds(ch * 512, 512)],
                                start=(j == 0), stop=(j == 2), perf_mode=DR)
                    for ch in range(2):
                        cols = bass.ds(ch * 512, 512)
                        pt = pts[ch]
                        MUL = mybir.AluOpType.mult
                        if m < 6:        # q rows (x256 prescale): heads 2m, 2m+1
                            nc.vector.tensor_scalar(
                                qhat[2 * m][64:128, cols], pt[0:64, :],
                                1.0 / 256.0, bcol[0:64, m:m + 1], MUL, ADD)
                            nc.vector.tensor_scalar(
                                qhat[2 * m + 1][64:128, cols], pt[64:128, :],
                                1.0 / 256.0, bcol[64:128, m:m + 1], MUL, ADD)
                        elif m < 12:     # k rows (x32): heads 2(m-6), 2(m-6)+1
                            nc.scalar.activation(
                                khat[2 * (m - 6)][64:128, cols], pt[0:64, :],
                                Ident, bias=bcol[0:64, m:m + 1], scale=1.0 / 32.0)
                            nc.scalar.activation(
                                khat[2 * (m - 6) + 1][64:128, cols], pt[64:128, :],
                                Ident, bias=bcol[64:128, m:m + 1], scale=1.0 / 32.0)
                        else:            # P rows (x256) -> shift-selector scatter
                            psb = psb_pool.tile([128, 512], BF16, tag="psb",
                                                name=f"psb_{m}_{ch}")
                            nc.scalar.activation(psb, pt, Ident,
                                                 bias=bcol[:, m:m + 1], scale=1.0 / 256.0)
                            # shift matmuls write 6 (hh, table) slots at
                            # 32-aligned PSUM rows across 2 banks; cols are
                            # dlt*64 + stream position.
                            pbA = pb_ps.tile([128, 512], F32, tag="pbA",
                                             name=f"pbA_{m}_{ch}")
                            pbB = pb_ps.tile([64, 512], F32, tag="pbB",
                                             name=f"pbB_{m}_{ch}")
                            # slot -> (tile, row): (hh, ti): ti 0=d, 1=w, 2=h
                            slots = {(0, 0): (pbA, 0), (0, 1): (pbA, 32),
                                     (0, 2): (pbA, 64), (1, 0): (pbA, 96),
                                     (1, 1): (pbB, 0), (1, 2): (pbB, 32)}
                            psb_r = psb.rearrange("p (a b w) -> p a b w", a=8, b=8)
                            psb_s = psb.rearrange("p (k d) -> p k d", d=8)
                            for hh in range(2):
                                for ti in range(3):
                                    pbt, r0 = slots[(hh, ti)]
                                    for dlt in range(8):
                                        if ti == 0:    # d: stream (b,w)
                                            rhs = psb[:, dlt * 64:(dlt + 1) * 64]
                                        elif ti == 1:  # w: stream (a,b)
                                            rhs = psb_s[:, :, dlt]
                                        else:          # h: stream (a,w)
                                            rhs = psb_r[:, :, dlt, :]
                                        tsel = (0, 2, 1)[ti]
                                        sc = (tsel * 8 + dlt) * 16 + hh * 8
                                        nc.tensor.matmul(
                                            pbt[r0:r0 + 8, dlt * 64:(dlt + 1) * 64],
                                            shsel[:, sc:sc + 8], rhs,
                                            start=(dlt == 0), stop=(dlt == 7),
                                            tile_position=(0, r0),
                                            skip_group_check=True)
                            # evacuate: d rows go straight to qhat (both
                            # ends 32-aligned); w+h go via one staging tile
                            # (permuted cols) then one identity DMA
                            for hh in range(2):
                                h = 2 * (m - 12) + hh
                                pbt_d, r_d = slots[(hh, 0)]
                                pbt_w, r_w = slots[(hh, 1)]
                                pbt_h, r_h = slots[(hh, 2)]
                                nc.vector.tensor_copy(qhat[h][0:8, cols],
                                                      pbt_d[r_d:r_d + 8, :])
                                stg = psb_pool.tile([8, 1024], BF16, tag="stg",
                                                    name=f"stg{hh}_{m}_{ch}")
                                # w: src (dlt, a, b) -> dst cols a*64+b*8+dlt
                                src_w = pbt_w[r_w:r_w + 8, :].rearrange(
                                    "p (d a b) -> p d a b", d=8, a=8)
                                dst_w = stg[:, 0:512].rearrange(
                                    "p (a b d) -> p d a b", a=8, b=8)
                                nc.vector.tensor_copy(dst_w, src_w)
                                # h: src (dlt, a, w) -> dst cols a*64+dlt*8+w
                                src_h = pbt_h[r_h:r_h + 8, :].rearrange(
                                    "p (d a w) -> p d a w", d=8, a=8)
                                dst_h = stg[:, 512:1024].rearrange(
                                    "p (a d w) -> p d a w", a=8, d=8)
                                if hh == 0:
                                    nc.vector.tensor_copy(dst_h, src_h)
                                else:
                                    nc.scalar.copy(dst_h, src_h)
                                # one DMA: stg halves -> qhat rows 8-15 / 16-23
                                qeng = nc.sync if hh == 0 else nc.gpsimd
                                qeng.dma_start(out=qhat[h][8:16, cols],
                                               in_=stg[:, 0:512])
                                qeng.dma_start(out=qhat[h][16:24, cols],
                                               in_=stg[:, 512:1024])

            def emit_attention(hp):
                for h in (2 * hp, 2 * hp + 1):
                    for wi in range(NWIN):
                        qcols = bass.ds(wi * 512, 512)
                        ptp = []
                        for pp in range(2):
                            ptile = pt_pool.tile([128, 1024], F8, tag="pt")
                            for kk in range(2):
                                kt = pp * 2 + kk
                                ps = s_ps.tile([128, 512], F32, tag="sps")
                                nc.tensor.matmul(
                                    ps, khat[h][:, wi * 512 + kt * 128: wi * 512 + (kt + 1) * 128],
                                    qhat[h][:, qcols], start=True, stop=True)
                                nc.scalar.activation(ptile[:, kk * 512:(kk + 1) * 512],
                                                     ps, Exp)
                            ptp.append(ptile)
                        pc = c_ps.tile([128, 512], F32, tag="cps")
                        for pp in range(2):
                            vv = vhat[wi * 2 + pp].rearrange(
                                "p (ko hc) -> p ko hc", ko=2, hc=VKO)
                            nc.tensor.matmul(
                                pc, vv[:, :, h * VSLOT:(h + 1) * VSLOT],
                                ptp[pp].rearrange("p (ko n) -> p ko n", ko=2),
                                start=(pp == 0), stop=(pp == 1),
                                perf_mode=DR)
                        # 1/den on DVE (fast NR approx, den at partition 0 -
                        # custom-DVE ops cannot partition-shift), broadcast
                        # on gpsimd into partitions 0-95 so the STT reads
                        # in0/in1 at the same base 32
                        if FAST_RECIP:
                            rec = dn_pool.tile([1, 512], F32, tag="rec")
                            nc.vector.reciprocal_approx_fast(rec, pc[0:1, :])
                            bb = dn_pool.tile([128, 512], F32, tag="bb")
                            nc.gpsimd.partition_broadcast(bb, rec)
                            bbv = bb[64:128, :]
                        else:
                            rec = dn_pool.tile([1, 512], BF16, tag="rec")
                            nc.vector.reciprocal(rec, pc[0:1, :])
                            pb64 = aux["r_ps"].tile([64, 512], F32, tag="rps")
                            nc.tensor.matmul(pb64, aux["ones_r64"], rec,
                                             start=True, stop=True)
                            bb = dn_pool.tile([64, 512], BF16, tag="bb")
                            nc.scalar.copy(bb, pb64)
                            bbv = bb
                        cdst = ctxv[h // 4][(h % 2) * 64:(h % 2) * 64 + 64,
                                            (h // 2) % 2, qcols]
                        nc.vector.scalar_tensor_tensor(
                            out=cdst, in0=pc[64:128, :], scalar=1.0,
                            in1=bbv, op0=mybir.AluOpType.mult,
                            op1=mybir.AluOpType.mult)

            if PIPELINE:
                for hp in range(7):
                    if hp < 6:
                        emit_qkv_scatter(hp)
                    if hp > 0:
                        emit_attention(hp - 1)
            else:
                for hp in range(6):
                    emit_qkv_scatter(hp)
                    emit_attention(hp)

        xh_cm.__exit__(None, None, None)
        vh_cm.__exit__(None, None, None)
        kh_cm.__exit__(None, None, None)
        qh_cm.__exit__(None, None, None)
        wq_cm.__exit__(None, None, None)

        # MLP weights: loaded now (space freed by attention pools); the
        # transfers overlap proj + LN2
        w1_cm = tc.tile_pool(name="w1", bufs=3)
        w1_pool = w1_cm.__enter__()
        w1s = [w1_pool.tile([128, 2 * 3072], F8, tag="w1", name=f"w1_{i}") for i in range(3)]
        w1v = [t.rearrange("p (ko m) -> p ko m", ko=2) for t in w1s]
        w2_cm = tc.tile_pool(name="w2", bufs=24)
        w2_pool = w2_cm.__enter__()
        w2s = [w2_pool.tile([128, 768], BF16, tag="w2", name=f"w2_{i}") for i in range(24)]
        for j in range(3):
            nc.sync.dma_start(out=w1s[j], in_=d["w1"][:, j * 6144:(j + 1) * 6144])
        for k in range(24):
            nc.sync.dma_start(out=w2s[k], in_=d["w2"][k * 128:(k + 1) * 128, :])

        # ---- proj + residual -> x2 (fp32) ----
        x2_cm = tc.tile_pool(name="x2", bufs=KT)
        x2_pool = x2_cm.__enter__()
        x2 = [x2_pool.tile([128, NTOK], BF16, tag="x2", name=f"x2_{i}") for i in range(KT)]
        with ExitStack() as ph:
            p_ps = ph.enter_context(tc.tile_pool(name="p_ps", bufs=4, space="PSUM"))
            for ch in range(2):
                cols = bass.ds(ch * 512, 512)
                for m in range(KT):
                    pp = p_ps.tile([128, 512], F32, tag="pps")
                    for g in range(3):
                        nc.tensor.matmul(pp, wpv[:, g, :, m * 128:(m + 1) * 128],
                                         ctxv[g][:, :, cols],
                                         start=(g == 0), stop=(g == 2),
                                         perf_mode=DR)
                    nc.vector.scalar_tensor_tensor(
                        out=x2[m][:, cols], in0=pp, scalar=1.0 / 1024.0,
                        in1=xs[m][:, cols], op0=mybir.AluOpType.mult, op1=ADD)

        # ---- LN2 -> mh (bf16) ----
        mh_cm = tc.tile_pool(name="mh", bufs=3)
        mh_pool = mh_cm.__enter__()
        mh = [mh_pool.tile([128, 2 * NTOK], F8, tag="mh", name=f"mh{i}") for i in range(3)]
        mhv = [t.rearrange("p (ko n) -> p ko n", ko=2) for t in mh]

        def ln2_out(k, ch, sb, bc_rs):
            eng = nc.gpsimd if k % 2 == 0 else nc.vector
            eng.tensor_mul(mhv[k // 2][:, k % 2, bass.ds(ch * 512, 512)], sb, bc_rs)
        _emit_ln(nc, tc, x2, ln2_out, consts)

        # ---- fc1 + gelu -> h1 (bf16) ----
        h1_cm = tc.tile_pool(name="h1", bufs=24)
        h1_pool = h1_cm.__enter__()
        h1 = [h1_pool.tile([128, NTOK], BF16, tag="h1", name=f"h1_{i}") for i in range(24)]
        with ExitStack() as ph:
            f1_ps = ph.enter_context(tc.tile_pool(name="f1_ps", bufs=6, space="PSUM"))
            gtmp = ph.enter_context(tc.tile_pool(name="gtmp", bufs=4)) if SIM_GELU else None
            for m in range(24):
                pfs = [f1_ps.tile([128, 512], F32, tag="f1ps",
                                  name=f"f1ps_{m}_{ch}") for ch in range(2)]
                for j in range(3):
                    for ch in range(2):
                        nc.tensor.matmul(
                            pfs[ch], w1v[j][:, :, m * 128:(m + 1) * 128],
                            mhv[j][:, :, ch * 512:(ch + 1) * 512],
                            start=(j == 0), stop=(j == 2), perf_mode=DR)
                for ch in range(2):
                    pf = pfs[ch]
                    h1dst = h1[m][:, ch * 512:(ch + 1) * 512]
                    if SIM_GELU:
                        xb = gtmp.tile([128, 512], BF16, tag="xb")
                        nc.scalar.activation(xb, pf, Ident,
                                             bias=b1[:, m:m + 1], scale=1.0 / 32.0)
                        sg = gtmp.tile([128, 512], BF16, tag="sg")
                        nc.scalar.activation(sg, xb, Sigmoid, scale=1.702)
                        nc.vector.tensor_mul(h1dst, xb, sg)
                    else:
                        nc.scalar.activation(h1dst, pf, Gelu,
                                             bias=b1[:, m:m + 1], scale=1.0 / 32.0)

        # ---- fc2 + residual -> out ----
        with ExitStack() as ph:
            f2_ps = ph.enter_context(tc.tile_pool(name="f2_ps", bufs=4, space="PSUM"))
            o_pool = ph.enter_context(tc.tile_pool(name="outT", bufs=2))
            for m in range(KT):
                ot = o_pool.tile([128, NTOK], F32, tag="ot")
                pfs = [f2_ps.tile([128, 512], F32, tag="f2ps",
                                  name=f"f2ps_{m}_{ch}") for ch in range(2)]
                for g in range(24):
                    for ch in range(2):
                        nc.tensor.matmul(
                            pfs[ch], w2s[g][:, m * 128:(m + 1) * 128],
                            h1[g][:, ch * 512:(ch + 1) * 512],
                            start=(g == 0), stop=(g == 23))
                for ch in range(2):
                    nc.vector.scalar_tensor_tensor(
                        out=ot[:, ch * 512:(ch + 1) * 512],
                        in0=pfs[ch], scalar=b2[:, m:m + 1],
                        in1=x2[m][:, ch * 512:(ch + 1) * 512],
                        op0=ADD, op1=ADD)
                    nc.sync.dma_start(
                        out=d["outT"][m * 128:(m + 1) * 128, ch * 512:(ch + 1) * 512],
                        in_=ot[:, ch * 512:(ch + 1) * 512])

        h1_cm.__exit__(None, None, None)
        mh_cm.__exit__(None, None, None)
        x2_cm.__exit__(None, None, None)
        w2_cm.__exit__(None, None, None)
        w1_cm.__exit__(None, None, None)
        ctx_cm.__exit__(None, None, None)


def _build(loop_n=None):
    nc = bacc.Bacc("TRN2", target_bir_lowering=False, debug=False, num_devices=8)
    dd = {}

    dd["xT"] = nc.dram_tensor("xT", [DIM, NTOK], BF16, kind="ExternalInput").ap()
    dd["bcol"] = nc.dram_tensor("bcol", [128, 18], F32, kind="ExternalInput").ap()
    dd["b1"] = nc.dram_tensor("b1", [128, 24], F32, kind="ExternalInput").ap()
    dd["b2"] = nc.dram_tensor("b2", [128, 6], F32, kind="ExternalInput").ap()
    dd["waug"] = nc.dram_tensor("waug", [128, 3 * 2 * 2304], F8, kind="ExternalInput").ap()
    dd["wv"] = nc.dram_tensor("wv", [128, 3 * 2 * 768], F8, kind="ExternalInput").ap()
    dd["wp"] = nc.dram_tensor("wp", [128, 3 * 2 * 768], F8, kind="ExternalInput").ap()
    dd["w1"] = nc.dram_tensor("w1", [128, 3 * 2 * 3072], F8, kind="ExternalInput").ap()
    dd["w2"] = nc.dram_tensor("w2", [3072, DIM], BF16, kind="ExternalInput").ap()
    dd["bv"] = nc.dram_tensor("bv", [1, DIM], BF16, kind="ExternalInput").ap()
    dd["e2"] = nc.dram_tensor("e2", [64, NTOK], BF16, kind="ExternalInput").ap()
    dd["shsel"] = nc.dram_tensor("shsel", [128, 384], BF16, kind="ExternalInput").ap()
    dd["outT"] = nc.dram_tensor("outT", [DIM, NTOK], F32, kind="ExternalOutput").ap()

    with tile.TileContext(nc) as tc:
        if loop_n is None:
            _emit(nc, tc, dd)
        else:
            with tc.For_i(0, loop_n, 1):
                _emit(nc, tc, dd)
    nc.compile()
    return nc


# ---------------------------------------------------------------------------
# host side
# ---------------------------------------------------------------------------

def _col_tiles(b):
    """(n*128,) bias -> (128, n) column-tile layout."""
    n = b.shape[0] // 128
    return np.ascontiguousarray(b.reshape(n, 128).T)


def _to_f8(a):
    return np.asarray(a, np.float32).astype(ml_dtypes.float8_e4m3)


def _dr_pack(A):
    """[768 ch, M] -> [128, 3*2*M] fp8 DoubleRow layout:
    channel c = 256j + 128ko + p  ->  [p, (j, ko, m)]."""
    C, M = A.shape
    assert C == 768
    B = A.reshape(3, 2, 128, M).transpose(2, 0, 1, 3).reshape(128, -1)
    return np.ascontiguousarray(B).astype(ml_dtypes.float8_e4m3)


def prep_weights(inputs):
    g = {k: np.asarray(v, np.float32) for k, v in inputs.items()}
    qkv_w, qkv_b = g["qkv_w"], g["qkv_b"]
    ln1_w, ln1_b = g["ln1_w"], g["ln1_b"]
    Wf = qkv_w * ln1_w[None, :]
    bf = qkv_b + qkv_w @ ln1_b
    Wq, bq = Wf[0:768], bf[0:768]
    Wk, bk = Wf[768:1536], bf[768:1536]
    Wv, bv = Wf[1536:2304], bf[1536:2304]
    rel = (g["rel_pos_d"], g["rel_pos_h"], g["rel_pos_w"])
    W_aug = np.zeros((2304, 768), np.float32)
    b_aug = np.zeros((2304,), np.float32)
    W_aug[0:768] = Wq * SCALE
    b_aug[0:768] = bq * SCALE
    W_aug[768:1536] = Wk
    b_aug[768:1536] = bk
    for h in range(NH):
        Wq_h, bq_h = Wq[h * 64:(h + 1) * 64], bq[h * 64:(h + 1) * 64]
        for ti in range(3):
            T = rel[ti][::-1]
            rows = 1536 + h * 64 + ti * 15
            W_aug[rows:rows + 15] = T @ Wq_h
            b_aug[rows:rows + 15] = T @ bq_h
    m = np.arange(512)
    # khat one-hot rows: 0-7 e_d, 8-15 e_w, 16-23 e_h, zeros elsewhere
    # (matches qhat bias rows: d 0-7, w 8-15, h 16-23)
    E = np.zeros((64, 512), np.float32)
    E[0 + m // 64, m] = 1.0
    E[8 + m % 8, m] = 1.0
    E[16 + (m // 8) % 8, m] = 1.0
    # shift-selector one-hots: for (ti, dlt), 16 output rows (j + 8*hh)
    # selecting psb row hh*64 + ti*15 + 7 - dlt + j
    shsel = np.zeros((128, 3 * 8 * 16), np.float32)
    for ti in range(3):
        for dlt in range(8):
            for hh in range(2):
                for j in range(8):
                    r = hh * 64 + ti * 15 + 7 - dlt + j
                    shsel[r, (ti * 8 + dlt) * 16 + hh * 8 + j] = 1.0
    # fp8 prescales: q/P rows x256, k rows x32 (descaled at evacuation);
    # wv/w1 x32. Values land in e4m3's normal range instead of subnormals.
    W_pre = W_aug.copy()
    W_pre[0:768] *= 256.0
    W_pre[768:1536] *= 32.0
    W_pre[1536:2304] *= 256.0
    return {
        "shsel": np.ascontiguousarray(shsel).astype(ml_dtypes.bfloat16),
        "waug": _dr_pack(W_pre.T),
        "bcol": _col_tiles(b_aug),
        "wv": _dr_pack(Wv.T * 32.0),
        "bv": np.ascontiguousarray(32.0 * bv[None, :]).astype(ml_dtypes.bfloat16),
        "wp": np.ascontiguousarray(
            (g["proj_w"].T * 64.0).reshape(3, 2, 128, 768)
            .transpose(2, 0, 1, 3).reshape(128, -1)).astype(ml_dtypes.float8_e4m3),
        "w1": _dr_pack((g["fc1_w"] * g["ln2_w"][None, :]).T * 32.0),
        "b1": _col_tiles(g["fc1_b"] + g["fc1_w"] @ g["ln2_b"]),
        "w2": np.ascontiguousarray(g["fc2_w"].T).astype(ml_dtypes.bfloat16),
        "b2": _col_tiles(g["fc2_b"]),
        "e2": np.ascontiguousarray(np.concatenate([E, E], axis=1)).astype(ml_dtypes.bfloat16),
    }


def shard_x(x):
    """x (B,D,H,W,C) -> list of 8 per-core (768, 1024) bf16 C-layout arrays."""
    B, D, H, W, C = x.shape
    win = x.reshape(B, D // WS, WS, H // WS, WS, W // WS, WS, C)
    win = win.transpose(0, 1, 3, 5, 2, 4, 6, 7).reshape(-1, WS ** 3, C)
    return [np.ascontiguousarray(win[2 * c:2 * c + 2].reshape(NTOK, C).T).astype(ml_dtypes.bfloat16)
            for c in range(8)]


def unshard_out(outs, shape):
    B, D, H, W, C = shape
    full = np.concatenate([o.T for o in outs], axis=0).reshape(16, WS ** 3, C)
    x = full.reshape(B, D // WS, H // WS, W // WS, WS, WS, WS, C)
    x = x.transpose(0, 1, 4, 2, 5, 3, 6, 7).reshape(B, D, H, W, C)
    return np.ascontiguousarray(x)


_STATE = {}


def _make_runner(nc):
    """Wrap a compiled Bass program in a persistent jitted SPMD callable."""
    import jax
    from jax.sharding import Mesh, PartitionSpec
    from jax.experimental.shard_map import shard_map
    from concourse import bass2jax

    bass2jax.install_neuronx_cc_hook()

    n_cores = 8
    partition_name = nc.partition_id_tensor.name if nc.partition_id_tensor else None
    in_names, out_names, out_avals, zero_outs = [], [], [], []
    for alloc in nc.m.functions[0].allocations:
        if not isinstance(alloc, mybir.MemoryLocationSet):
            continue
        name = alloc.memorylocations[0].name
        if alloc.kind == "ExternalInput":
            if name != partition_name:
                in_names.append(name)
        elif alloc.kind == "ExternalOutput":
            out_names.append(name)
            shape = tuple(alloc.tensor_shape)
            dtype = mybir.dt.np(alloc.dtype)
            out_avals.append(jax.core.ShapedArray(shape, dtype))
            zero_outs.append(np.zeros(shape, dtype))
    n_params = len(in_names)
    all_in_names = in_names + out_names
    if partition_name is not None:
        all_in_names = all_in_names + [partition_name]

    def _body(*args):
        operands = list(args)
        if partition_name is not None:
            operands.append(bass2jax.partition_id_tensor())
        outs = bass2jax._bass_exec_p.bind(
            *operands,
            out_avals=tuple(out_avals),
            in_names=tuple(all_in_names),
            out_names=tuple(out_names),
            lowering_input_output_aliases=(),
            sim_require_finite=True,
            sim_require_nnan=True,
            nc=nc,
        )
        return tuple(outs)

    devices = jax.devices()[:n_cores]
    mesh = Mesh(np.asarray(devices), ("core",))
    donate = tuple(range(n_params, n_params + len(out_names)))
    sharded = jax.jit(
        shard_map(_body, mesh=mesh,
                  in_specs=(PartitionSpec("core"),) * (n_params + len(out_names)),
                  out_specs=(PartitionSpec("core"),) * len(out_names)),
        donate_argnums=donate, keep_unused=True)

    def run(in_maps):
        per_core = [[np.asarray(m[nm]) for nm in in_names] for m in in_maps]
        concat_in = [np.concatenate([per_core[c][i] for c in range(n_cores)], axis=0)
                     for i in range(n_params)]
        concat_zero = [np.zeros((n_cores * z.shape[0], *z.shape[1:]), z.dtype)
                       for z in zero_outs]
        out_arrs = sharded(*concat_in, *concat_zero)
        return [
            {nm: np.asarray(out_arrs[i]).reshape(n_cores, *out_avals[i].shape)[c]
             for i, nm in enumerate(out_names)}
            for c in range(n_cores)
        ]

    return run, dict(sharded=sharded, body=_body, in_names=in_names,
                     out_names=out_names, out_avals=out_avals,
                     zero_outs=zero_outs, mesh=mesh, n_params=n_params)


def _get_runner():
    if "run" not in _STATE:
        run, internals = _make_runner(_build())
        _STATE["run"] = run
        _STATE["internals"] = internals
    return _STATE["run"]


def kernel(**inputs):
    x = np.asarray(inputs["x"], np.float32)
    w = prep_weights(inputs)
    shards = shard_x(x)
    in_maps = [dict(w, xT=shards[c]) for c in range(8)]
    run = _get_runner()
    results = run(in_maps)
    outs = [results[c]["outT"] for c in range(8)]
    return unshard_out(outs, x.shape)

